# revision 1
# baseline (speedup 1.0000x reference)
"""CostVolumeLayer3D Trainium2 kernel (final).

Product-bound design point: DVE tensor_tensor at ~2 elem/lane/cycle is the
architectural wall (~384us/core), so v5 minimizes DVE instruction count and
keeps every other engine off the critical path.

- Host pre-builds FIVE x-shifted, halo-free x2 variants (xb = 0..4, width
  64, y-stride 64). On device, (y,x) merge into one contiguous AP dim, so
  one TT per (diagonal, y-block) covers all 4 t-slices x 5 d-shifts via an
  overlapping (t, dd) access pattern: 72 TT ops total (vs 180), all
  4B-aligned (no odd-shift copies, no ACT prep).
- One-hot reduction matmuls: [128, 30] lhsT per shift into three 32-aligned
  PSUM col-groups (cheap LDWEIGHTS, col-group overlap), 20 matmuls per TT.
- ACT extracts rows 0..94 with the 1/125 scale; host reassembles the 45
  surviving channels of the 125-channel output.

Sharding: depth D=32 -> 4 output slices per core (8 cores), halo-padded
x2 shards. Partitions = (b, c) = 2*64 = 128.
"""

import numpy as np

_B, _C, _D, _H, _W = 2, 64, 32, 64, 64
_R = 2
_NCH = 125
_RNG = 2 * _R + 1
_NCORES = 8
_DL = _D // _NCORES          # 4
_DH = _DL + 2 * _R           # 8
_YB = 8                      # y-block rows
_YHH = _YB + 2 * _R          # 12
_HP = _H + 2 * _R            # 68 padded y rows
_NG = 3                      # psum col groups
_GW = 15                     # shifts per group
_GROWS = 2 * _GW             # 30


def _shift_table():
    shifts = []
    for sd in range(-4, 5):
        i = min(2, sd + 2)
        j = sd - i
        for h in range(-2, 3):
            shifts.append(((5 * sd + h) % _NCH, _R - h, _R - i, _R - j))
    return shifts


_SHIFTS = _shift_table()
_NS = len(_SHIFTS)           # 45


def _ones_lhst(np_dt):
    a = np.zeros((_NS, 128, _GROWS), dtype=np_dt)
    for s in range(_NS):
        i = s % _GW
        a[s, 0:64, 2 * i] = 1.0
        a[s, 64:128, 2 * i + 1] = 1.0
    return a


_prog = None


def _build_program():
    global _prog
    if _prog is not None:
        return _prog
    from contextlib import ExitStack

    import concourse.bacc as bacc
    import concourse.mybir as mybir
    import concourse.tile as tile
    from concourse.ap import AP

    f16 = mybir.dt.float16
    f32 = mybir.dt.float32
    nc = bacc.Bacc(trn_type="TRN2", debug=False)
    x1_d = nc.dram_tensor("x1", [_B, _C, _DL, _H, _W], f16, kind="ExternalInput")
    # five x-shifted halo-free variants of the padded x2 shard
    x2_d = nc.dram_tensor(
        "x2v", [_RNG, _B, _C, _DH, _HP, _W], f16, kind="ExternalInput"
    )
    on_d = nc.dram_tensor("ones", [_NS, 128, _GROWS], f16, kind="ExternalInput")
    out_d = nc.dram_tensor(
        "out", [_NG, _GW, _B, _DL, _H, _W], f32, kind="ExternalOutput"
    )

    nfree = _YB * _W          # 512
    dstride = _YHH * _W       # variant tile d stride (768 elems)

    with tile.TileContext(nc) as tc:
        with ExitStack() as ctx:
            constp = ctx.enter_context(tc.tile_pool(name="const", bufs=1))
            x2p = ctx.enter_context(tc.tile_pool(name="x2v", bufs=2))
            x1p = ctx.enter_context(tc.tile_pool(name="x1", bufs=2))
            prodp = ctx.enter_context(tc.tile_pool(name="prod", bufs=3))
            psump = ctx.enter_context(tc.tile_pool(name="psum", bufs=2, space="PSUM"))
            stagep = ctx.enter_context(tc.tile_pool(name="stage", bufs=6))

            ones_t = constp.tile([128, _NS, _GROWS], f16)
            nc.sync.dma_start(ones_t[:], on_d.ap().rearrange("s k m -> k s m"))

            for yi in range(_H // _YB):
                y0 = yi * _YB
                x1_t = x1p.tile([128, _DL, _YB, _W], f16, tag="x1")
                nc.sync.dma_start(
                    x1_t[:],
                    x1_d.ap()[:, :, :, y0 : y0 + _YB, :].rearrange(
                        "b c t y x -> (b c) t y x"
                    ),
                )
                x2_t = x2p.tile([128, _RNG, _DH, _YHH, _W], f16, tag="x2v")
                for v in (4, 3, 2, 1, 0):
                    nc.sync.dma_start(
                        x2_t[:, v],
                        x2_d.ap()[v, :, :, :, y0 : y0 + _YHH, :].rearrange(
                            "b c d y x -> (b c) d y x"
                        ),
                    )
                x1_b = x1_t[:].unsqueeze(2).broadcast_to([128, _DL, _RNG, _YB, _W])

                pss = [
                    psump.tile([96, nfree], f32, tag=f"ps{t}", name=f"ps{t}")
                    for t in range(_DL)
                ]
                started = [[False] * _NG for _ in range(_DL)]
                nmm = [[0] * _NG for _ in range(_DL)]
                for di in range(_NS // _RNG):
                    _ch, _dd0, yy0, xx0 = _SHIFTS[_RNG * di]
                    base = x2_t[:, xx0]
                    ov = AP(
                        base.tensor,
                        base.offset + yy0 * _W,
                        [
                            list(base.ap[0]),
                            [dstride, _DL],
                            [dstride, _RNG],
                            [1, nfree],
                        ],
                    )
                    pr = prodp.tile([128, _DL, _RNG, nfree], f16, tag="pr")
                    nc.vector.tensor_mul(
                        pr[:], x1_b.rearrange("p t h y x -> p t h (y x)"), ov
                    )
                    for t in range(_DL):
                        for q in range(_RNG):
                            s = _RNG * di + (_RNG - 1 - q)
                            g = s // _GW
                            nmm[t][g] += 1
                            nc.tensor.matmul(
                                pss[t][32 * g : 32 * g + _GROWS, :],
                                lhsT=ones_t[:, s, :],
                                rhs=pr[:, t, q, :],
                                start=not started[t][g],
                                stop=nmm[t][g] == _GW,
                                tile_position=(0, 32 * g),
                            )
                            started[t][g] = True
                    # group di//3 just completed: stream it out now
                    if di % 3 == 2:
                        g = di // 3
                        for t in range(_DL):
                            st = stagep.tile([_GROWS, nfree], f32, tag="st")
                            nc.scalar.mul(
                                st[:], pss[t][32 * g : 32 * g + _GROWS, :], 1.0 / _NCH
                            )
                            nc.sync.dma_start(
                                out_d.ap()[g, :, :, t, y0 : y0 + _YB, :].rearrange(
                                    "i b y x -> (i b) (y x)"
                                ),
                                st[:],
                            )
    nc.compile()
    _prog = nc
    return nc


def _shard_inputs(x1, x2):
    x2pad = np.pad(
        np.asarray(x2), ((0, 0), (0, 0), (_R, _R), (_R, _R), (_R, _R))
    ).astype(np.float16)
    x1 = np.asarray(x1)
    ones_np = _ones_lhst(np.float16)
    in_maps = []
    for k in range(_NCORES):
        d0 = k * _DL
        shard = x2pad[:, :, d0 : d0 + _DH]           # [B, C, DH, HP, W+4]
        x2v = np.stack(
            [shard[:, :, :, :, xb : xb + _W] for xb in range(_RNG)]
        )                                             # [5, B, C, DH, HP, W]
        in_maps.append(
            {
                "x1": np.ascontiguousarray(x1[:, :, d0 : d0 + _DL].astype(np.float16)),
                "x2v": np.ascontiguousarray(x2v),
                "ones": ones_np,
            }
        )
    return in_maps


def _gather(results):
    out = np.zeros((_B, _NCH, _D, _H, _W), dtype=np.float32)
    for k in range(_NCORES):
        o = results[k]["out"]  # [NG, GW, B, DL, H, W]
        d0 = k * _DL
        for s, (ch, _dd0, _yy0, _xx0) in enumerate(_SHIFTS):
            out[:, ch, d0 : d0 + _DL] = o[s // _GW, s % _GW]
    return out


def _run(in_maps, **kwargs):
    from concourse.bass_utils import run_bass_kernel_spmd

    nc = _build_program()
    return run_bass_kernel_spmd(nc, in_maps, core_ids=list(range(_NCORES)), **kwargs)


def kernel(**inputs):
    res = _run(_shard_inputs(inputs["x1"], inputs["x2"]))
    return _gather(res.results)



# revision 3
# speedup vs baseline: 1.5323x; 1.5323x over previous
"""CostVolumeLayer3D Trainium2 kernel v5: PE-Gram with per-piy column tiling.

v4 -> v5: each brick's Gram runs as FOUR concurrent col-tile matmuls
(tile_position (0, 32*piy), K=128 block-diag over batch, M=32), each
streaming only its piy's 5-row y-window (24 cells x 8 d = 192 columns).
The y-shift index becomes partition-uniform, so the dumped gram shrinks
384 -> 192 columns (25 -> 12.6 MB), halving copy and dump cost. Inputs
are issued via the idle GPSIMD's SWDGE so dumps never queue behind them.
"""

from contextlib import ExitStack

import numpy as np

_B, _C, _D, _H, _W = 2, 64, 32, 64, 64
_R = 2
_NCH = 125
_NCORES = 8
_DL = _D // _NCORES          # 4
_BK = 4
_NBY = _H // _BK             # 16
_NBX = _W // _BK             # 16
_NBLK = _NBY * _NBX          # 256
_DH = _DL + 2 * _R           # 8
_HP = _H + 2 * _R            # 68
_WP = _W + 2 * _R            # 68
_NA = 5 * 4 * _DH            # 160 region-A cols per piy (yy0 5, xrel 4, d 8)
_NB = 4 * _DH                # 32 region-B cols per piy (xrel 4, d 8)
_NQ = _NA + _NB              # 192
_GRP = 8                     # bricks per dump DMA
_NSTR = 4                    # input stripes
_SY = 20


def _shift_table():
    shifts = []
    for sd in range(-4, 5):
        i = min(2, sd + 2)
        j = sd - i
        for h in range(-2, 3):
            shifts.append((((5 * sd + h) % _NCH), _R - h, _R - i, _R - j))
    return shifts


_SHIFTS = _shift_table()
_NS = len(_SHIFTS)

_prog = None


def _build_program():
    global _prog
    if _prog is not None:
        return _prog

    import concourse.bacc as bacc
    import concourse.mybir as mybir
    import concourse.tile as tile

    f16 = mybir.dt.float16
    f32 = mybir.dt.float32
    nc = bacc.Bacc(trn_type="TRN2", debug=False)
    # block-diag lhsT per (brick, piy): [blk, piy, (b c), m=b*16+pd*4+px]
    x1_d = nc.dram_tensor("x1c", [128, _NBLK, _BK, 32], f16, kind="ExternalInput")
    x2_d = nc.dram_tensor(
        "x2h", [_NSTR, _B * _C, _SY, _WP, _DH], f16, kind="ExternalInput"
    )
    g_d = nc.dram_tensor("gram", [128, _NBLK, _NQ], f16, kind="ExternalOutput")

    with tile.TileContext(nc) as tc:
        with ExitStack() as ctx:
            x2p = ctx.enter_context(tc.tile_pool(name="x2", bufs=1))
            x1p = ctx.enter_context(tc.tile_pool(name="x1", bufs=1))
            psump = ctx.enter_context(tc.tile_pool(name="ps", bufs=8, space="PSUM"))
            stagep = ctx.enter_context(tc.tile_pool(name="st", bufs=5))

            x2s = []
            x1s = []
            nb4 = _NBLK // _NSTR
            for g in range(_NSTR):
                # all inputs ordered on the HWDGE sync queue: stripe 0
                # loads first at full bandwidth; gram dumps go out via
                # GPSIMD SWDGE so they never queue behind the inputs.
                eng = nc.sync
                x2t = x2p.tile([128, _SY, _WP, _DH], f16, tag=f"x2s{g}", name=f"x2s{g}")
                eng.dma_start(x2t[:], x2_d.ap()[g])
                x2s.append(x2t)
                x1t = x1p.tile([128, nb4, _BK, 32], f16, tag=f"x1s{g}", name=f"x1s{g}")
                eng.dma_start(x1t[:], x1_d.ap()[:, g * nb4 : (g + 1) * nb4])
                x1s.append(x1t)

            for grp in range(_NBLK // _GRP):
                st = stagep.tile([128, _GRP, _NQ], f16, tag="st")
                for g2 in range(_GRP // 2):
                    ps = psump.tile([128, 2, _NQ], f32, tag="ps")
                    for bi2 in range(2):
                        blk = grp * _GRP + g2 * 2 + bi2
                        yi, xi = divmod(blk, _NBX)
                        stripe = yi // 4
                        x2t = x2s[stripe]
                        x1t = x1s[stripe]
                        ly0 = _BK * yi - 16 * stripe
                        lblk = blk % nb4
                        x0 = _BK * xi
                        for py in range(_BK):
                            lhs = x1t[:, lblk, py, :]
                            rhs_a = x2t[:, ly0 + py : ly0 + py + 5, x0 + 4 : x0 + 8, :]
                            rhs_b = x2t[:, ly0 + py, x0 : x0 + 4, :]
                            nc.tensor.matmul(
                                ps[32 * py : 32 * py + 32, bi2, 0:_NA],
                                lhsT=lhs,
                                rhs=rhs_a,
                                start=True,
                                stop=True,
                                tile_position=(0, 32 * py),
                            )
                            nc.tensor.matmul(
                                ps[32 * py : 32 * py + 32, bi2, _NA:_NQ],
                                lhsT=lhs,
                                rhs=rhs_b,
                                start=True,
                                stop=True,
                                tile_position=(0, 32 * py),
                            )
                    b0 = g2 * 2
                    if g2 % 2 == 0:
                        nc.vector.tensor_copy(st[:, b0 : b0 + 2, :], ps[:])
                    else:
                        nc.scalar.copy(st[:, b0 : b0 + 2, :], ps[:])
                nc.gpsimd.dma_start(
                    g_d.ap()[:, grp * _GRP : (grp + 1) * _GRP, :], st[:]
                )
    nc.compile()
    _prog = nc
    return nc


def _shard_inputs(x1, x2):
    x1f = (np.asarray(x1, np.float32) * (1.0 / _NCH)).astype(np.float16)
    x2f = np.asarray(x2, np.float32).astype(np.float16)
    x2pad = np.pad(x2f, ((0, 0), (0, 0), (_R, _R), (_R, _R), (_R, _R)))
    in_maps = []
    for k in range(_NCORES):
        d0 = k * _DL
        slab = x1f[:, :, d0 : d0 + _DL]             # [B,C,4,64,64]
        # -> [blk, piy, (b c), b*16+pd*4+px] block-diag over b
        x1c = np.zeros((128, _NBLK, _BK, 32), np.float16)
        r = slab.reshape(_B, _C, _BK, _NBY, _BK, _NBX, _BK)  # b c pd yi py xi px
        r = r.transpose(0, 1, 3, 5, 4, 2, 6)          # b c yi xi py pd px
        r = r.reshape(_B, _C, _NBLK, _BK, _BK * _BK)  # b c blk py (pd px)
        for b in range(_B):
            x1c[b * 64 : (b + 1) * 64, :, :, b * 16 : (b + 1) * 16] = r[b]
        x2h = np.ascontiguousarray(
            x2pad[:, :, d0 : d0 + _DH].transpose(0, 1, 3, 4, 2)
        ).reshape(_B * _C, _HP, _WP, _DH)
        x2str = np.stack([x2h[:, 16 * g : 16 * g + _SY] for g in range(_NSTR)])
        in_maps.append(
            {
                "x1c": np.ascontiguousarray(x1c),
                "x2h": np.ascontiguousarray(x2str),
            }
        )
    return in_maps


_IDX_CACHE = None


def _gather_indices():
    global _IDX_CACHE
    if _IDX_CACHE is not None:
        return _IDX_CACHE
    d = np.arange(_D)
    y = np.arange(_H)
    x = np.arange(_W)
    core = (d // _DL)[:, None, None]
    blk = (y // _BK)[:, None] * _NBX + (x // _BK)[None, :]
    pd = (d % _BK)[:, None, None]
    py = (y % _BK)[None, :, None]
    px = (x % _BK)[None, None, :]
    # partition row: 32*py + b*16 + pd*4 + px  (b added in _gather)
    m_pos = 32 * py + pd * 4 + px
    qs = np.empty((_NS, _D, _H, _W), np.int64)
    chans = np.empty(_NS, np.int64)
    for s, (ch, dd0, yy0, xx0) in enumerate(_SHIFTS):
        chans[s] = ch
        dpp = pd + dd0                                # [32,1,1]
        if xx0 == 4:
            q = (yy0 * 4 + px) * _DH + dpp            # region A, xrel=px
        else:  # yy0 == 0
            xpp = px + xx0
            q = np.where(
                xpp >= 4,
                ((xpp - 4)) * _DH + dpp,              # A with yy0=0
                _NA + xpp * _DH + dpp,                # region B
            )
        qs[s] = np.broadcast_to(q, (_D, _H, _W))
    _IDX_CACHE = (core, blk, m_pos, qs, chans)
    return _IDX_CACHE


def _gather(results):
    core, blk, m_pos, qs, chans = _gather_indices()
    gram = np.stack([np.asarray(results[k]["gram"]) for k in range(_NCORES)])
    out = np.zeros((_B, _NCH, _D, _H, _W), np.float32)
    for b in range(_B):
        m = m_pos + b * 16
        vals = gram[core[None], m[None], blk[None, None], qs].astype(np.float32)
        out[b, chans] = vals
    return out


def _run(in_maps, **kwargs):
    from concourse.bass_utils import run_bass_kernel_spmd

    nc = _build_program()
    return run_bass_kernel_spmd(nc, in_maps, core_ids=list(range(_NCORES)), **kwargs)


def kernel(**inputs):
    res = _run(_shard_inputs(inputs["x1"], inputs["x2"]))
    return _gather(res.results)


# revision 4
# speedup vs baseline: 1.5487x; 1.0107x over previous
"""CostVolumeLayer3D Trainium2 kernel v5: PE-Gram with per-piy column tiling.

v4 -> v5: each brick's Gram runs as FOUR concurrent col-tile matmuls
(tile_position (0, 32*piy), K=128 block-diag over batch, M=32), each
streaming only its piy's 5-row y-window (24 cells x 8 d = 192 columns).
The y-shift index becomes partition-uniform, so the dumped gram shrinks
384 -> 192 columns (25 -> 12.6 MB), halving copy and dump cost. Inputs
are issued via the idle GPSIMD's SWDGE so dumps never queue behind them.
"""

from contextlib import ExitStack

import numpy as np

_B, _C, _D, _H, _W = 2, 64, 32, 64, 64
_R = 2
_NCH = 125
_NCORES = 8
_DL = _D // _NCORES          # 4
_BK = 4
_NBY = _H // _BK             # 16
_NBX = _W // _BK             # 16
_NBLK = _NBY * _NBX          # 256
_DH = _DL + 2 * _R           # 8
_HP = _H + 2 * _R            # 68
_WP = _W + 2 * _R            # 68
_NA = 5 * 4 * _DH            # 160 region-A cols per piy (yy0 5, xrel 4, d 8)
_NB = 4 * _DH                # 32 region-B cols per piy (xrel 4, d 8)
_NQ = _NA + _NB              # 192
_GRP = 8                     # bricks per dump DMA
_NSTR = 4                    # input stripes
_SY = 20


def _shift_table():
    shifts = []
    for sd in range(-4, 5):
        i = min(2, sd + 2)
        j = sd - i
        for h in range(-2, 3):
            shifts.append((((5 * sd + h) % _NCH), _R - h, _R - i, _R - j))
    return shifts


_SHIFTS = _shift_table()
_NS = len(_SHIFTS)

_prog = None


def _build_program():
    global _prog
    if _prog is not None:
        return _prog

    import concourse.bacc as bacc
    import concourse.mybir as mybir
    import concourse.tile as tile

    f16 = mybir.dt.float16
    f32 = mybir.dt.float32
    nc = bacc.Bacc(trn_type="TRN2", debug=False)
    # block-diag lhsT per (brick, piy): [blk, piy, (b c), m=b*16+pd*4+px]
    x1_d = nc.dram_tensor("x1c", [128, _NBLK, _BK, 32], f16, kind="ExternalInput")
    x2_d = nc.dram_tensor(
        "x2h", [_NSTR, _B * _C, _SY, _WP, _DH], f16, kind="ExternalInput"
    )
    g_d = nc.dram_tensor("gram", [128, _NBLK, _NQ], f16, kind="ExternalOutput")

    with tile.TileContext(nc) as tc:
        with ExitStack() as ctx:
            x2p = ctx.enter_context(tc.tile_pool(name="x2", bufs=1))
            x1p = ctx.enter_context(tc.tile_pool(name="x1", bufs=1))
            psump = ctx.enter_context(tc.tile_pool(name="ps", bufs=8, space="PSUM"))
            stagep = ctx.enter_context(tc.tile_pool(name="st", bufs=5))

            x2s = []
            x1s = []
            nb4 = _NBLK // _NSTR
            for g in range(_NSTR):
                # all inputs ordered on the HWDGE sync queue: stripe 0
                # loads first at full bandwidth; gram dumps go out via
                # GPSIMD SWDGE so they never queue behind the inputs.
                eng = nc.sync
                x2t = x2p.tile([128, _SY, _WP, _DH], f16, tag=f"x2s{g}", name=f"x2s{g}")
                eng.dma_start(x2t[:], x2_d.ap()[g])
                x2s.append(x2t)
                x1t = x1p.tile([128, nb4, _BK, 32], f16, tag=f"x1s{g}", name=f"x1s{g}")
                eng.dma_start(x1t[:], x1_d.ap()[:, g * nb4 : (g + 1) * nb4])
                x1s.append(x1t)

            for grp in range(_NBLK // _GRP):
                st = stagep.tile([128, _GRP, _NQ], f16, tag="st")
                for g2 in range(_GRP // 2):
                    ps = psump.tile([128, 2, _NQ], f32, tag="ps")
                    for bi2 in range(2):
                        blk = grp * _GRP + g2 * 2 + bi2
                        yi, xi = divmod(blk, _NBX)
                        stripe = yi // 4
                        x2t = x2s[stripe]
                        x1t = x1s[stripe]
                        ly0 = _BK * yi - 16 * stripe
                        lblk = blk % nb4
                        x0 = _BK * xi
                        for py in range(_BK):
                            lhs = x1t[:, lblk, py, :]
                            rhs_a = x2t[:, ly0 + py : ly0 + py + 5, x0 + 4 : x0 + 8, :]
                            rhs_b = x2t[:, ly0 + py, x0 : x0 + 4, :]
                            nc.tensor.matmul(
                                ps[32 * py : 32 * py + 32, bi2, 0:_NA],
                                lhsT=lhs,
                                rhs=rhs_a,
                                start=True,
                                stop=True,
                                tile_position=(0, 32 * py),
                            )
                            nc.tensor.matmul(
                                ps[32 * py : 32 * py + 32, bi2, _NA:_NQ],
                                lhsT=lhs,
                                rhs=rhs_b,
                                start=True,
                                stop=True,
                                tile_position=(0, 32 * py),
                            )
                    b0 = g2 * 2
                    if g2 % 2 == 0:
                        nc.vector.tensor_copy(st[:, b0 : b0 + 2, :], ps[:])
                    else:
                        nc.scalar.copy(st[:, b0 : b0 + 2, :], ps[:])
                # tail groups dump via the (by-then idle) sync HWDGE queue:
                # faster completion chain than SWDGE at the kernel tail.
                deng = nc.sync if grp >= 28 else nc.gpsimd
                deng.dma_start(
                    g_d.ap()[:, grp * _GRP : (grp + 1) * _GRP, :], st[:]
                )
    nc.compile()
    _prog = nc
    return nc


def _shard_inputs(x1, x2):
    x1f = (np.asarray(x1, np.float32) * (1.0 / _NCH)).astype(np.float16)
    x2f = np.asarray(x2, np.float32).astype(np.float16)
    x2pad = np.pad(x2f, ((0, 0), (0, 0), (_R, _R), (_R, _R), (_R, _R)))
    in_maps = []
    for k in range(_NCORES):
        d0 = k * _DL
        slab = x1f[:, :, d0 : d0 + _DL]             # [B,C,4,64,64]
        # -> [blk, piy, (b c), b*16+pd*4+px] block-diag over b
        x1c = np.zeros((128, _NBLK, _BK, 32), np.float16)
        r = slab.reshape(_B, _C, _BK, _NBY, _BK, _NBX, _BK)  # b c pd yi py xi px
        r = r.transpose(0, 1, 3, 5, 4, 2, 6)          # b c yi xi py pd px
        r = r.reshape(_B, _C, _NBLK, _BK, _BK * _BK)  # b c blk py (pd px)
        for b in range(_B):
            x1c[b * 64 : (b + 1) * 64, :, :, b * 16 : (b + 1) * 16] = r[b]
        x2h = np.ascontiguousarray(
            x2pad[:, :, d0 : d0 + _DH].transpose(0, 1, 3, 4, 2)
        ).reshape(_B * _C, _HP, _WP, _DH)
        x2str = np.stack([x2h[:, 16 * g : 16 * g + _SY] for g in range(_NSTR)])
        in_maps.append(
            {
                "x1c": np.ascontiguousarray(x1c),
                "x2h": np.ascontiguousarray(x2str),
            }
        )
    return in_maps


_IDX_CACHE = None


def _gather_indices():
    global _IDX_CACHE
    if _IDX_CACHE is not None:
        return _IDX_CACHE
    d = np.arange(_D)
    y = np.arange(_H)
    x = np.arange(_W)
    core = (d // _DL)[:, None, None]
    blk = (y // _BK)[:, None] * _NBX + (x // _BK)[None, :]
    pd = (d % _BK)[:, None, None]
    py = (y % _BK)[None, :, None]
    px = (x % _BK)[None, None, :]
    # partition row: 32*py + b*16 + pd*4 + px  (b added in _gather)
    m_pos = 32 * py + pd * 4 + px
    qs = np.empty((_NS, _D, _H, _W), np.int64)
    chans = np.empty(_NS, np.int64)
    for s, (ch, dd0, yy0, xx0) in enumerate(_SHIFTS):
        chans[s] = ch
        dpp = pd + dd0                                # [32,1,1]
        if xx0 == 4:
            q = (yy0 * 4 + px) * _DH + dpp            # region A, xrel=px
        else:  # yy0 == 0
            xpp = px + xx0
            q = np.where(
                xpp >= 4,
                ((xpp - 4)) * _DH + dpp,              # A with yy0=0
                _NA + xpp * _DH + dpp,                # region B
            )
        qs[s] = np.broadcast_to(q, (_D, _H, _W))
    _IDX_CACHE = (core, blk, m_pos, qs, chans)
    return _IDX_CACHE


def _gather(results):
    core, blk, m_pos, qs, chans = _gather_indices()
    gram = np.stack([np.asarray(results[k]["gram"]) for k in range(_NCORES)])
    out = np.zeros((_B, _NCH, _D, _H, _W), np.float32)
    for b in range(_B):
        m = m_pos + b * 16
        vals = gram[core[None], m[None], blk[None, None], qs].astype(np.float32)
        out[b, chans] = vals
    return out


def _run(in_maps, **kwargs):
    from concourse.bass_utils import run_bass_kernel_spmd

    nc = _build_program()
    return run_bass_kernel_spmd(nc, in_maps, core_ids=list(range(_NCORES)), **kwargs)


def kernel(**inputs):
    res = _run(_shard_inputs(inputs["x1"], inputs["x2"]))
    return _gather(res.results)
